# revision 1
# baseline (speedup 1.0000x reference)
"""EntityPredictionHead_CandidateList Trainium2 kernel (v2).

Math (full inputs):
    h = LayerNorm(gelu(hidden_states @ W_dense.T + b_dense)) * ln_gamma + ln_beta
    scores[b, c] = dot(decoder_table[cand_idx[b, c]], h[b]) + entity_bias[cand_idx[b, c]]

Sharding: batch (4096) split across 8 NeuronCores (512 rows each); the
bf16 decoder table is split into 16 residue sub-tables (rows v % 16 == r)
replicated per core.

Per-core device pipeline:
  1. Transform on PE/ACT/DVE -> h^T (bf16, [emb x 512], two 128-partition
     e-halves) entirely in SBUF.  hidden^T and W^T arrive pre-transposed
     from the host, so the only on-chip transposes are 8 PE 128x128 tiles
     of the normalized output.
  2. The 65536 candidate positions are bucketed by v mod 16 on the host and
     sorted by batch row within each bucket.  One dma_gather per bucket
     (transpose=True) pulls the bf16 table rows from that residue's
     sub-table into an emb-major SBUF tile G^T [128, 2, CAP_r]; idx = v//16
     fits int16 because each sub-table has only 31250 rows.
  3. Because slots are batch-sorted, the batch rows of any 128-slot chunk
     fit a static 64-wide window W (derived from the data on the host).
     Two PE matmuls per chunk compute M = G^T.T @ h^T[:, W:W+64] -- every
     slot's dot product against all 64 window rows, accumulated over the
     two e-halves in PSUM.
  4. ACT copies M to SBUF (bf16).  DVE builds the one-hot selector
     oh[s, d] = (b_local[s] == d) via is_equal against a static iota row
     (batch index rides as the per-partition scalar; no broadcasts), then
     multiplies and tensor-reduces -> one score column per chunk.
  5. Host un-permutes scores back to (b, c) order and adds the bias gather.
"""

import numpy as np
import ml_dtypes

import concourse.bacc as bacc
import concourse.mybir as mybir
import concourse.tile as tile
from concourse.bass_utils import run_bass_kernel_spmd
from concourse.masks import make_identity
from concourse.tile import add_dep_helper

# Problem shape (hardcoded per contract).
B = 4096
HIDDEN = 1024
EMB = 256
VOCAB = 500000
C = 128
EPS = 1e-12

N_CORES = 8
B_LOC = B // N_CORES           # 512 batch rows per core
P = 128
NBB = B_LOC // P               # 4 batch blocks per core
NPOS = B_LOC * C               # 65536 gather positions per core
NRES = 16                      # residue classes (v mod 16)
RTAB = VOCAB // NRES           # 31250 rows per residue sub-table
WIN = 64                       # batch-window width per 128-slot chunk
N_QUEUES = 4
# SWDGE descriptor ring: 128 descs per DMA engine per direction; a
# transposed 512B-row gather needs num_idxs/8 + 2 RX descs per engine ->
# num_idxs <= 1008. Stay at 896 for margin.
MAXG = 896


def _split_cap(cap):
    """Split cap into multiples of 128, each <= MAXG."""
    out = []
    rem = cap
    while rem > 0:
        take = min(MAXG, rem)
        out.append(take)
        rem -= take
    return out

F32 = mybir.dt.float32
BF16 = mybir.dt.bfloat16
I16 = mybir.dt.int16

BF = ml_dtypes.bfloat16


def build_program(caps, wins):
    """caps: tuple of 16 slot capacities (multiples of 128).
    wins: tuple of 16 tuples of per-chunk window starts (ints in [0, 448])."""
    nch = sum(c // P for c in caps)
    nidx16 = sum(c // 16 for c in caps)

    nc = bacc.Bacc(None, num_swdge_queues=N_QUEUES)

    hidT = nc.dram_tensor("hidT", [HIDDEN, B_LOC], BF16, kind="ExternalInput")
    wT = nc.dram_tensor("wT", [HIDDEN, EMB], BF16, kind="ExternalInput")
    bde = nc.dram_tensor("bde", [1, EMB], BF16, kind="ExternalInput")
    gmb = nc.dram_tensor("gmb", [P, 4], F32, kind="ExternalInput")  # gamma cols | beta cols
    iota = nc.dram_tensor("iota", [P, WIN], BF16, kind="ExternalInput")
    tabs = [nc.dram_tensor(f"tab{r}", [RTAB, EMB], BF16, kind="ExternalInput")
            for r in range(NRES)]
    vidx = nc.dram_tensor("vidx", [P, nidx16], I16, kind="ExternalInput")
    bloc = nc.dram_tensor("bloc", [P, nch], BF16, kind="ExternalInput")
    scores = nc.dram_tensor("scores", [P, nch], F32, kind="ExternalOutput")

    with tile.TileContext(nc) as tc:
        with (
            tc.tile_pool(name="persist", bufs=1) as persist,
            tc.tile_pool(name="tf", bufs=2) as tfp,
            tc.tile_pool(name="small", bufs=1) as smallp,
            tc.tile_pool(name="gather", bufs=4) as gpool,
            tc.tile_pool(name="msb", bufs=2) as mpool,
            tc.tile_pool(name="oh", bufs=2) as ohpool,
            tc.tile_pool(name="prod", bufs=2) as prodpool,
            tc.tile_pool(name="psum_tf", bufs=2, space="PSUM") as psum_tf,
            tc.tile_pool(name="psum_t", bufs=2, space="PSUM") as psum_t,
            tc.tile_pool(name="psum_m", bufs=3, space="PSUM") as psum_m,
        ):
            # ---- prologue ----
            ident = persist.tile([P, P], F32)
            make_identity(nc, ident[:])

            ones_bf = persist.tile([1, P], BF16)
            nc.vector.memset(ones_bf[:], 1.0)

            eps_col = persist.tile([P, 1], F32)
            nc.vector.memset(eps_col[:], EPS)

            bde_sb = persist.tile([1, EMB], BF16)
            nc.sync.dma_start(bde_sb[:], bde[:])
            gmb_sb = persist.tile([P, 4], F32)
            nc.sync.dma_start(gmb_sb[:], gmb[:])
            iota_sb = persist.tile([P, WIN], BF16)
            nc.sync.dma_start(iota_sb[:], iota[:])

            hidT_sb = persist.tile([P, (HIDDEN // P) * B_LOC], BF16)
            for kb in range(HIDDEN // P):
                nc.sync.dma_start(hidT_sb[:, kb * B_LOC:(kb + 1) * B_LOC],
                                  hidT[kb * P:(kb + 1) * P, :])
            wT_sb = persist.tile([P, (HIDDEN // P) * EMB], BF16)
            for kb in range(HIDDEN // P):
                nc.sync.dma_start(wT_sb[:, kb * EMB:(kb + 1) * EMB],
                                  wT[kb * P:(kb + 1) * P, :])

            vidx_sb = persist.tile([P, nidx16], I16)
            nc.sync.dma_start(vidx_sb[:], vidx[:])
            bloc_sb = persist.tile([P, nch], BF16)
            nc.sync.dma_start(bloc_sb[:], bloc[:])

            # h^T e-halves, bf16 [128 emb, 512 batch]
            hT0 = persist.tile([P, B_LOC], BF16)
            hT1 = persist.tile([P, B_LOC], BF16)
            hT = [hT0, hT1]

            # ---- transform: h = LN(gelu(hidden @ W.T + b)) -> h^T ----
            for bb in range(NBB):
                ph = psum_tf.tile([P, EMB], F32, space="PSUM", tag="ph")
                for kb in range(HIDDEN // P):
                    nc.tensor.matmul(
                        ph[:],
                        lhsT=hidT_sb[:, kb * B_LOC + bb * P:kb * B_LOC + (bb + 1) * P],
                        rhs=wT_sb[:, kb * EMB:(kb + 1) * EMB],
                        start=(kb == 0), stop=False,
                    )
                nc.tensor.matmul(ph[:], lhsT=ones_bf[:], rhs=bde_sb[:],
                                 start=False, stop=True)

                g_sb = tfp.tile([P, EMB], F32, tag="g")
                sum_g = smallp.tile([P, 1], F32, tag="sumg")
                nc.scalar.activation(g_sb[:], ph[:],
                                     mybir.ActivationFunctionType.Gelu,
                                     accum_out=sum_g[:])
                mu = smallp.tile([P, 1], F32, tag="mu")
                nc.scalar.mul(mu[:], sum_g[:], 1.0 / EMB)

                cent = tfp.tile([P, EMB], F32, tag="cent")
                nc.vector.tensor_scalar(cent[:], g_sb[:], mu[:], None,
                                        mybir.AluOpType.subtract)

                sq_trash = tfp.tile([P, EMB], F32, tag="sqt")
                ssq = smallp.tile([P, 1], F32, tag="ssq")
                nc.scalar.activation(sq_trash[:], cent[:],
                                     mybir.ActivationFunctionType.Square,
                                     accum_out=ssq[:])
                std = smallp.tile([P, 1], F32, tag="std")
                nc.scalar.activation(std[:], ssq[:],
                                     mybir.ActivationFunctionType.Sqrt,
                                     bias=eps_col[:, 0:1], scale=1.0 / EMB)
                rstd = smallp.tile([P, 1], F32, tag="rstd")
                nc.vector.reciprocal(rstd[:], std[:])

                hs = tfp.tile([P, EMB], F32, tag="hs")
                nc.vector.tensor_scalar(hs[:], cent[:], rstd[:], None,
                                        mybir.AluOpType.mult)

                for g in range(2):
                    pt = psum_t.tile([P, P], F32, space="PSUM", tag="tp")
                    nc.tensor.transpose(pt[:], hs[:, g * P:(g + 1) * P], ident[:])
                    # gamma * x + beta, written bf16 into the e-half tile
                    nc.vector.tensor_scalar(
                        hT[g][:, bb * P:(bb + 1) * P], pt[:],
                        gmb_sb[:, g:g + 1], gmb_sb[:, 2 + g:3 + g],
                        mybir.AluOpType.mult, mybir.AluOpType.add)

            # ---- gather + score per residue ----
            sc_sb = persist.tile([P, nch], F32)
            prev_gather = None

            def _chain(bass_inst, sync=False):
                nonlocal prev_gather
                inst = bass_inst.ins
                if prev_gather is not None:
                    add_dep_helper(inst, prev_gather, sync=sync,
                                   reason="pin SWDGE sem-lane/queue pairing")
                prev_gather = inst
                return bass_inst

            # Warmup: one small gather per SWDGE queue, fully consumed before
            # any real gather issues.  The first transposed gather on a queue
            # intermittently returns partial data when it races the prologue
            # DMA burst; these absorb the first-use window.
            wtrash = persist.tile([P, P], F32)
            wacc = persist.tile([P, 1], F32)
            warm_consumers = []
            for wq in range(N_QUEUES):
                wg = gpool.tile([P, 2 * P], BF16, tag="warm")
                _chain(nc.gpsimd.dma_gather(
                    wg[:].rearrange("p (g i) -> p g i", i=P),
                    tabs[wq][:], vidx_sb[:, 0:P // 16],
                    P, P, EMB, transpose=True, queue_num=wq))
                cons = nc.scalar.activation(
                    wtrash[:], wg[:, 0:P],
                    mybir.ActivationFunctionType.Copy, accum_out=wacc[:])
                warm_consumers.append(cons.ins)

            voff = 0
            col0 = 0
            gcount = 0
            for r in range(NRES):
                cap = caps[r]
                q_r = cap // P
                subs = _split_cap(cap)

                m_sb = mpool.tile([P, q_r * WIN], BF16)
                c_base = 0
                for sub in subs:
                    gt = gpool.tile([P, 2 * sub], BF16, tag="gt")
                    gv = gt[:].rearrange("p (g i) -> p g i", i=sub)
                    ginst = _chain(nc.gpsimd.dma_gather(
                        gv, tabs[r][:],
                        vidx_sb[:, voff:voff + sub // 16],
                        sub, sub, EMB,
                        transpose=True,
                        queue_num=gcount % N_QUEUES,
                    ))
                    if gcount == 0:
                        for wc in warm_consumers:
                            add_dep_helper(ginst.ins, wc, sync=True,
                                           reason="warmup before real gathers")
                    voff += sub // 16
                    gcount += 1

                    nsub_c = sub // P
                    for g8 in range((nsub_c * WIN + 511) // 512):
                        w = min(512, nsub_c * WIN - g8 * 512)
                        mp = psum_m.tile([P, 512], F32, space="PSUM", tag="m")
                        for cl in range(g8 * (512 // WIN),
                                        g8 * (512 // WIN) + w // WIN):
                            off = (cl - g8 * (512 // WIN)) * WIN
                            wq = wins[r][c_base + cl]
                            for g in range(2):
                                nc.tensor.matmul(
                                    mp[:, off:off + WIN],
                                    lhsT=gv[:, g, cl * P:(cl + 1) * P],
                                    rhs=hT[g][:, wq:wq + WIN],
                                    start=(g == 0), stop=(g == 1),
                                )
                        nc.scalar.copy(
                            m_sb[:, (c_base * WIN + g8 * 512):
                                 (c_base * WIN + g8 * 512 + w)],
                            mp[:, :w])
                    c_base += nsub_c

                oh = ohpool.tile([P, q_r * WIN], BF16)
                ohv = oh[:].rearrange("p (q d) -> p q d", d=WIN)
                nc.vector.tensor_tensor(
                    ohv,
                    iota_sb[:].unsqueeze(1).broadcast_to((P, q_r, WIN)),
                    bloc_sb[:, col0:col0 + q_r].unsqueeze(2).broadcast_to(
                        (P, q_r, WIN)),
                    op=mybir.AluOpType.is_equal)

                prod = prodpool.tile([P, q_r * WIN], BF16)
                nc.vector.tensor_tensor(prod[:], m_sb[:], oh[:],
                                        op=mybir.AluOpType.mult)
                nc.vector.tensor_reduce(
                    sc_sb[:, col0:col0 + q_r],
                    prod[:].rearrange("p (q d) -> p q d", d=WIN),
                    axis=mybir.AxisListType.X,
                    op=mybir.AluOpType.add)
                col0 += q_r

            nc.sync.dma_start(scores[:], sc_sb[:])

    nc.compile()
    return nc


_NC_CACHE = {}


def _get_program(caps, wins):
    key = (caps, wins)
    if key not in _NC_CACHE:
        _NC_CACHE.clear()
        _NC_CACHE[key] = build_program(caps, wins)
    return _NC_CACHE[key]


def _wrap_idx(vals):
    """[N] int16 array (N % 16 == 0) -> [128, N//16]: position i at
    (partition i%16, col i//16), replicated across the 8 16-partition
    groups."""
    n = len(vals)
    w = vals.reshape(n // 16, 16).T  # [16, n//16]
    return np.tile(w, (8, 1)).astype(np.int16)


def make_in_maps(hidden_states, W_dense, b_dense, ln_gamma, ln_beta,
                 decoder_table, entity_bias, cand_idx):
    hidden_states = np.asarray(hidden_states, dtype=np.float32)
    cand = np.asarray(cand_idx)
    table_bf = np.asarray(decoder_table, dtype=np.float32).astype(BF)
    tabs = {f"tab{r}": np.ascontiguousarray(table_bf[r::NRES])
            for r in range(NRES)}

    wT_h = np.ascontiguousarray(np.asarray(W_dense, dtype=np.float32).T
                                ).astype(BF)
    gmb = np.empty((P, 4), dtype=np.float32)
    gmb[:, 0] = np.asarray(ln_gamma, dtype=np.float32)[:P]
    gmb[:, 1] = np.asarray(ln_gamma, dtype=np.float32)[P:]
    gmb[:, 2] = np.asarray(ln_beta, dtype=np.float32)[:P]
    gmb[:, 3] = np.asarray(ln_beta, dtype=np.float32)[P:]
    iota = np.tile(np.arange(WIN, dtype=np.float32), (P, 1)).astype(BF)
    bde = np.asarray(b_dense, dtype=np.float32).reshape(1, EMB).astype(BF)

    # --- per-core residue bucketing (sorted by batch row) ---
    # per core & residue: original flat positions, sorted by batch row
    order = []   # order[core][r] = int array of flat positions
    for core in range(N_CORES):
        v = cand[core * B_LOC:(core + 1) * B_LOC].reshape(-1).astype(np.int64)
        b = np.arange(NPOS, dtype=np.int64) // C
        res = v % NRES
        per_r = []
        for r in range(NRES):
            pos = np.nonzero(res == r)[0]
            pos = pos[np.argsort(b[pos], kind="stable")]
            per_r.append(pos)
        order.append(per_r)

    caps = tuple(
        int(-(-max(len(order[c][r]) for c in range(N_CORES)) // P) * P)
        for r in range(NRES))

    # --- static per-chunk windows (shared across cores) ---
    wins = []
    for r in range(NRES):
        q_r = caps[r] // P
        w_r = []
        for q in range(q_r):
            lo, hi = B_LOC, -1
            for core in range(N_CORES):
                pos = order[core][r][q * P:(q + 1) * P]
                if len(pos):
                    bseg = pos // C
                    lo = min(lo, int(bseg.min()))
                    hi = max(hi, int(bseg.max()))
            if hi < 0:
                lo, hi = 0, 0
            assert hi - lo + 1 <= WIN, (r, q, lo, hi)
            w_r.append(min(lo, B_LOC - WIN))
        wins.append(tuple(w_r))
    wins = tuple(wins)
    nch = sum(c // P for c in caps)

    in_maps = []
    perms = []
    for core in range(N_CORES):
        v = cand[core * B_LOC:(core + 1) * B_LOC].reshape(-1).astype(np.int64)
        vidx_parts = []
        bloc = np.zeros((P, nch), dtype=np.float32)
        # perm[col, p] = original flat position (or -1 for padding)
        perm = np.full((nch, P), -1, dtype=np.int64)
        col0 = 0
        for r in range(NRES):
            cap = caps[r]
            pos = order[core][r]
            n = len(pos)
            idx = np.zeros(cap, dtype=np.int16)
            idx[:n] = (v[pos] // NRES).astype(np.int16)
            s0 = 0
            for sub in _split_cap(cap):
                vidx_parts.append(_wrap_idx(idx[s0:s0 + sub]))
                s0 += sub
            q_r = cap // P
            for q in range(q_r):
                seg = pos[q * P:(q + 1) * P]
                if len(seg):
                    bl = seg // C - wins[r][q]
                    bloc[:len(seg), col0 + q] = bl
                    perm[col0 + q, :len(seg)] = seg
            col0 += q_r
        in_maps.append({
            "hidT": np.ascontiguousarray(
                hidden_states[core * B_LOC:(core + 1) * B_LOC].T).astype(BF),
            "wT": wT_h,
            "bde": bde,
            "gmb": gmb,
            "iota": iota,
            **tabs,
            "vidx": np.ascontiguousarray(np.hstack(vidx_parts)),
            "bloc": bloc.astype(BF),
        })
        perms.append(perm)
    global _LAST_AUX
    _LAST_AUX = (perms, caps, wins)
    return in_maps, _LAST_AUX


_LAST_AUX = None


def run(in_maps, trace=False, aux=None):
    if aux is None:
        aux = _LAST_AUX
    assert aux is not None, "call make_in_maps first"
    _perms, caps, wins = aux
    nc = _get_program(caps, wins)
    return run_bass_kernel_spmd(nc, in_maps, core_ids=list(range(N_CORES)),
                                trace=trace)


def unpermute_scores(raw, perm):
    """raw: device scores [128, nch]; perm: [nch, 128] original flat
    positions (-1 = padding). Return [B_LOC, C]."""
    flat = raw.T.reshape(-1)          # [(col, p)]
    pf = perm.reshape(-1)
    out = np.empty(NPOS, dtype=raw.dtype)
    m = pf >= 0
    out[pf[m]] = flat[m]
    return out.reshape(B_LOC, C)


def kernel(hidden_states, W_dense, b_dense, ln_gamma, ln_beta,
           decoder_table, entity_bias, cand_idx):
    in_maps, aux = make_in_maps(hidden_states, W_dense, b_dense, ln_gamma,
                                ln_beta, decoder_table, entity_bias, cand_idx)
    perms, caps, wins = aux
    res = run(in_maps, aux=aux)
    bias = np.asarray(entity_bias, dtype=np.float32)[np.asarray(cand_idx)]
    parts = [unpermute_scores(np.asarray(res.results[i]["scores"],
                                         dtype=np.float32), perms[i])
             for i in range(N_CORES)]
    return np.concatenate(parts, axis=0) + bias

